# revision 1
# baseline (speedup 1.0000x reference)
"""Binarized CNN (XNOR-style) inference kernel for Trainium2, 8 NeuronCores.

Strategy
--------
Data parallel: 16 images per core, weights replicated.

The network binarizes every activation/weight to +-1 after layer 1, so all
convs 2..7 are exact-integer arithmetic.  We exploit:
  * sign(clip(c*s+t)) == (c >= -t/s ? +1 : -1)  for s>0  -> BN+clip+sign folds
    into one compare-with-threshold per channel.
  * maxpool commutes with the monotone threshold compare -> pool raw conv sums.
  * +-1 values are exact in bf16 and conv sums are small integers, exact in
    fp32 PSUM accumulation -> bf16 matmuls are bit-exact.  We encode activations
    as +-0.5 (so a single fused (x>=thr)-0.5 tensor_scalar produces them); every
    layer's conv sums are then exactly half the reference's, compensated in the
    thresholds (t/(2s)) and the final BN scale (2*s7).
  * conv1 must be accurate to <1e-7 (the data has one element 1.09e-7 from its
    threshold).  We decompose x into 5 fixed-point bf16 planes (8 significant
    bits each, lsb 2^-4..2^-36) and run 3 matmuls (planes stacked pairwise in
    K: 54+54+27 rows).  Each matmul's fp32 PSUM accumulation is exact (fixed
    point windows fit 24 bits); the two inter-plane accumulates round only at
    the final value -> total error ~3e-8 near the decision boundary,
    deterministically matching the float64-faithful binarization.

Convs are implemented as implicit GEMM: activations live in SBUF as
[C_part, n, H+2, W+2] bf16 (zero halo); each 3x3 tap is one matmul with a
shifted 3-free-dim access pattern accumulating into a [128,512] PSUM tile.
"""

import numpy as np
import ml_dtypes

import concourse.bass as bass
import concourse.bacc as bacc_m
import concourse.tile as tile
import concourse.mybir as mybir
from concourse.bass_utils import run_bass_kernel_spmd

F32 = mybir.dt.float32
BF16 = mybir.dt.bfloat16
BF16_NP = ml_dtypes.bfloat16

NCORES = 8
NIMG = 16          # images per core
CHUNK = 2          # images per L1 input chunk
IS_GE = mybir.AluOpType.is_ge
SUB = mybir.AluOpType.subtract
ADD = mybir.AluOpType.add
MULT = mybir.AluOpType.mult

_CACHED_NC = None


def _build_program(debug_b1=False):
    nc = bacc_m.Bacc(None)

    XA = nc.declare_dram_parameter("xa", [54, NIMG, 34, 34], BF16, isOutput=False)
    XB = nc.declare_dram_parameter("xb", [54, NIMG, 34, 34], BF16, isOutput=False)
    XC = nc.declare_dram_parameter("xc", [27, NIMG, 34, 34], BF16, isOutput=False)
    W1D = nc.declare_dram_parameter("w1d", [54, 128], BF16, isOutput=False)
    W1S = nc.declare_dram_parameter("w1s", [27, 128], BF16, isOutput=False)
    W2 = nc.declare_dram_parameter("w2t", [128, 9, 128], BF16, isOutput=False)
    W3 = nc.declare_dram_parameter("w3t", [128, 2, 9, 128], BF16, isOutput=False)
    W4 = nc.declare_dram_parameter("w4t", [128, 2, 2, 9, 128], BF16, isOutput=False)
    W5 = nc.declare_dram_parameter("w5t", [128, 4, 2, 9, 128], BF16, isOutput=False)
    W6 = nc.declare_dram_parameter("w6t", [128, 4, 4, 9, 128], BF16, isOutput=False)
    W7 = nc.declare_dram_parameter("w7t", [128, 64, 10], BF16, isOutput=False)
    THR = nc.declare_dram_parameter("thr", [128, 14], F32, isOutput=False)
    BN7 = nc.declare_dram_parameter("bn7", [10, 2], F32, isOutput=False)
    OUT = nc.declare_dram_parameter("out", [NIMG, 10], F32, isOutput=True)
    if debug_b1:
        DBG = nc.declare_dram_parameter("dbg_b1", [128, NIMG, 34, 34], BF16,
                                        isOutput=True)

    with tile.TileContext(nc) as tc:
        with tc.tile_pool(name="w", bufs=1) as wp, \
             tc.tile_pool(name="b1p", bufs=1) as b1p, \
             tc.tile_pool(name="tmp", bufs=4) as tp, \
             tc.tile_pool(name="psum", bufs=6, space="PSUM") as pp:

            w1d = wp.tile([54, 128], BF16)
            w1s = wp.tile([27, 128], BF16)
            thr = wp.tile([128, 14], F32)
            bn7 = wp.tile([10, 2], F32)
            nc.scalar.dma_start(w1d[:], W1D[:])
            nc.scalar.dma_start(w1s[:], W1S[:])
            nc.scalar.dma_start(thr[:], THR[:])

            b1 = b1p.tile([128, NIMG, 34, 34], BF16)
            nc.vector.memset(b1[:, :, 0:34:33, :], 0.0)
            nc.vector.memset(b1[:, :, 1:33, 0:34:33], 0.0)

            # weight tiles (DMAs emitted after L1 so x-chunks win the queue)
            w2t = wp.tile([128, 9, 128], BF16)
            w3t = wp.tile([128, 2, 9, 128], BF16)
            w4t = wp.tile([128, 2, 2, 9, 128], BF16)
            w5t = wp.tile([128, 4, 2, 9, 128], BF16)
            w6t = wp.tile([128, 4, 4, 9, 128], BF16)
            w7t = wp.tile([128, 64, 10], BF16)

            # ---- L1: exact conv via 5 bf16 fixed-point planes (3 MMs) ----
            with tc.tile_pool(name="xp", bufs=2) as xp:
                for ch_i, c0 in enumerate(range(0, NIMG, CHUNK)):
                    xa_t = xp.tile([54, CHUNK, 34, 34], BF16, tag="xa")
                    xb_t = xp.tile([54, CHUNK, 34, 34], BF16, tag="xb")
                    xc_t = xp.tile([27, CHUNK, 34, 34], BF16, tag="xc")
                    nc.sync.dma_start(xa_t[:], XA[:, c0:c0 + CHUNK])
                    nc.gpsimd.dma_start(xb_t[:], XB[:, c0:c0 + CHUNK])
                    nc.gpsimd.dma_start(xc_t[:], XC[:, c0:c0 + CHUNK])
                    if ch_i == 0:
                        nc.scalar.dma_start(w2t[:], W2[:])
                    elif ch_i == 2:
                        nc.scalar.dma_start(w3t[:], W3[:])
                    elif ch_i == 4:
                        nc.scalar.dma_start(w4t[:], W4[:])
                    for ci in range(CHUNK):
                        n = c0 + ci
                        for h in range(2):
                            ps = pp.tile([128, 16, 32], F32, tag="acc")
                            nc.tensor.matmul(
                                ps[:], w1d[:],
                                xa_t[:, ci, 16 * h:16 * h + 16, 0:32],
                                start=True, stop=False)
                            nc.tensor.matmul(
                                ps[:], w1d[:],
                                xb_t[:, ci, 16 * h:16 * h + 16, 0:32],
                                start=False, stop=False)
                            nc.tensor.matmul(
                                ps[:], w1s[:],
                                xc_t[:, ci, 16 * h:16 * h + 16, 0:32],
                                start=False, stop=True)
                            nc.vector.tensor_scalar(
                                b1[:, n, 1 + 16 * h:17 + 16 * h, 1:33],
                                ps[:], thr[:, 0:1], 0.5, IS_GE, SUB)

            nc.scalar.dma_start(w5t[:], W5[:])
            nc.scalar.dma_start(w6t[:], W6[:])
            nc.scalar.dma_start(w7t[:], W7[:])
            nc.scalar.dma_start(bn7[:], BN7[:])

            with tc.tile_pool(name="bp", bufs=1) as bp:
                b2 = bp.tile([128, NIMG, 18, 18], BF16)
                b3 = [bp.tile([128, NIMG, 18, 18], BF16, tag=f"b3_{m}", name=f"b3_{m}")
                      for m in range(2)]
                b4 = [bp.tile([128, NIMG, 10, 10], BF16, tag=f"b4_{m}", name=f"b4_{m}")
                      for m in range(2)]
                b5 = [bp.tile([128, NIMG, 10, 10], BF16, tag=f"b5_{m}", name=f"b5_{m}")
                      for m in range(4)]
                b6 = [bp.tile([128, NIMG, 4, 4], BF16, tag=f"b6_{m}", name=f"b6_{m}")
                      for m in range(4)]
                for t_ in [b2] + b3 + b4 + b5:
                    hw = t_.shape[-1]
                    nc.vector.memset(t_[:, :, 0:hw:hw - 1, :], 0.0)
                    nc.vector.memset(t_[:, :, 1:hw - 1, 0:hw:hw - 1], 0.0)

                # ---- L2: 128->128, pool, 32x32 -> 16x16 ----
                for n in range(NIMG):
                    for h in range(2):
                        ps = pp.tile([128, 16, 32], F32, tag="acc")
                        for t9 in range(9):
                            dy, dx = divmod(t9, 3)
                            nc.tensor.matmul(
                                ps[:], w2t[:, t9, :],
                                b1[:, n, 16 * h + dy:16 * h + dy + 16,
                                   dx:dx + 32],
                                start=(t9 == 0), stop=(t9 == 8))
                        t1 = tp.tile([128, 16, 16], F32, tag="t21")
                        nc.vector.tensor_reduce(
                            t1[:], ps[:].rearrange(
                                "p y (x two) -> p y x two", two=2),
                            op=mybir.AluOpType.max, axis=mybir.AxisListType.X)
                        t2 = tp.tile([128, 8, 16], F32, tag="t22")
                        nc.vector.tensor_reduce(
                            t2[:], t1[:].rearrange(
                                "p (y two) x -> p y x two", two=2),
                            op=mybir.AluOpType.max, axis=mybir.AxisListType.X)
                        nc.vector.tensor_scalar(
                            b2[:, n, 1 + 8 * h:9 + 8 * h, 1:17],
                            t2[:], thr[:, 1:2], 0.5, IS_GE, SUB)

                # ---- L3: 128->256, 16x16 ----
                for n2 in range(NIMG // 2):
                    for m in range(2):
                        ps = pp.tile([128, 2, 16, 16], F32, tag="acc")
                        for t9 in range(9):
                            dy, dx = divmod(t9, 3)
                            nc.tensor.matmul(
                                ps[:], w3t[:, m, t9, :],
                                b2[:, 2 * n2:2 * n2 + 2, dy:dy + 16,
                                   dx:dx + 16],
                                start=(t9 == 0), stop=(t9 == 8))
                        nc.vector.tensor_scalar(
                            b3[m][:, 2 * n2:2 * n2 + 2, 1:17, 1:17],
                            ps[:], thr[:, 2 + m:3 + m], 0.5, IS_GE, SUB)

                # ---- L4: 256->256, pool, 16x16 -> 8x8 ----
                for n2 in range(NIMG // 2):
                    for m in range(2):
                        ps = pp.tile([128, 2, 16, 16], F32, tag="acc")
                        idx = 0
                        for kg in range(2):
                            for t9 in range(9):
                                dy, dx = divmod(t9, 3)
                                nc.tensor.matmul(
                                    ps[:], w4t[:, m, kg, t9, :],
                                    b3[kg][:, 2 * n2:2 * n2 + 2, dy:dy + 16,
                                           dx:dx + 16],
                                    start=(idx == 0), stop=(idx == 17))
                                idx += 1
                        t1 = tp.tile([128, 2, 16, 8], F32, tag="t41")
                        nc.vector.tensor_reduce(
                            t1[:].rearrange("p n y x -> p (n y) x"),
                            ps[:].rearrange(
                                "p n y (x two) -> p (n y) x two", two=2),
                            op=mybir.AluOpType.max, axis=mybir.AxisListType.X)
                        t2 = tp.tile([128, 2, 8, 8], F32, tag="t42")
                        nc.vector.tensor_reduce(
                            t2[:].rearrange("p n y x -> p (n y) x"),
                            t1[:].rearrange(
                                "p n (y two) x -> p (n y) x two", two=2),
                            op=mybir.AluOpType.max, axis=mybir.AxisListType.X)
                        nc.vector.tensor_scalar(
                            b4[m][:, 2 * n2:2 * n2 + 2, 1:9, 1:9],
                            t2[:], thr[:, 4 + m:5 + m], 0.5, IS_GE, SUB)

                # ---- L5: 256->512, 8x8 ----
                for n8 in range(2):
                    for m in range(4):
                        ps = pp.tile([128, 8, 8, 8], F32, tag="acc")
                        idx = 0
                        for kg in range(2):
                            for t9 in range(9):
                                dy, dx = divmod(t9, 3)
                                nc.tensor.matmul(
                                    ps[:], w5t[:, m, kg, t9, :],
                                    b4[kg][:, 8 * n8:8 * n8 + 8, dy:dy + 8,
                                           dx:dx + 8],
                                    start=(idx == 0), stop=(idx == 17))
                                idx += 1
                        nc.vector.tensor_scalar(
                            b5[m][:, 8 * n8:8 * n8 + 8, 1:9, 1:9],
                            ps[:], thr[:, 6 + m:7 + m], 0.5, IS_GE, SUB)

                # ---- L6: 512->512, pool, 8x8 -> 4x4 ----
                for n8 in range(2):
                    for m in range(4):
                        ps = pp.tile([128, 8, 8, 8], F32, tag="acc")
                        idx = 0
                        for kg in range(4):
                            for t9 in range(9):
                                dy, dx = divmod(t9, 3)
                                nc.tensor.matmul(
                                    ps[:], w6t[:, m, kg, t9, :],
                                    b5[kg][:, 8 * n8:8 * n8 + 8, dy:dy + 8,
                                           dx:dx + 8],
                                    start=(idx == 0), stop=(idx == 35))
                                idx += 1
                        t1 = tp.tile([128, 8, 8, 4], F32, tag="t61")
                        nc.vector.tensor_reduce(
                            t1[:].rearrange("p n y x -> p (n y) x"),
                            ps[:].rearrange(
                                "p n y (x two) -> p (n y) x two", two=2),
                            op=mybir.AluOpType.max, axis=mybir.AxisListType.X)
                        t2 = tp.tile([128, 8, 4, 4], F32, tag="t62")
                        nc.vector.tensor_reduce(
                            t2[:].rearrange("p n y x -> p (n y) x"),
                            t1[:].rearrange(
                                "p n (y two) x -> p (n y) x two", two=2),
                            op=mybir.AluOpType.max, axis=mybir.AxisListType.X)
                        nc.vector.tensor_scalar(
                            b6[m][:, 8 * n8:8 * n8 + 8, :, :],
                            t2[:], thr[:, 10 + m:11 + m], 0.5, IS_GE, SUB)

                # ---- L7: 512x4x4 -> 10 logits ----
                ps7 = pp.tile([10, 16], F32, tag="z7", bufs=1)
                idx = 0
                for kg in range(4):
                    for t16 in range(16):
                        dy, dx = divmod(t16, 4)
                        nc.tensor.matmul(
                            ps7[:], w7t[:, kg * 16 + t16, :],
                            b6[kg][:, :, dy, dx],
                            start=(idx == 0), stop=(idx == 63))
                        idx += 1

                # ---- BN7 + log_softmax ----
                sq = tp.tile([32, 32], F32, tag="sq")
                nc.vector.memset(sq[:], 0.0)
                nc.vector.tensor_scalar(sq[0:10, 0:16], ps7[:], bn7[:, 0:1],
                                     bn7[:, 1:2], MULT, ADD)
                tq = tp.tile([32, 32], F32, tag="tq")
                nc.vector.transpose(tq[:], sq[:])
                yt = tq[0:16, 0:10]
                nm = tp.tile([16, 1], F32, tag="nm")
                nc.vector.tensor_reduce(nm[:], yt, op=mybir.AluOpType.max,
                                        axis=mybir.AxisListType.X, negate=True)
                e = tp.tile([16, 10], F32, tag="e")
                nc.scalar.activation(e[:], yt,
                                     mybir.ActivationFunctionType.Exp,
                                     bias=nm[:], scale=1.0)
                S = tp.tile([16, 1], F32, tag="S")
                nc.vector.tensor_reduce(S[:], e[:], op=ADD,
                                        axis=mybir.AxisListType.X)
                lnS = tp.tile([16, 1], F32, tag="lnS")
                nc.scalar.activation(lnS[:], S[:],
                                     mybir.ActivationFunctionType.Ln)
                o = tp.tile([16, 10], F32, tag="o")
                nc.vector.tensor_scalar(o[:], yt, nm[:], lnS[:], ADD, SUB)
                nc.sync.dma_start(OUT[:], o[:])

            if debug_b1:
                nc.sync.dma_start(DBG[:], b1[:])

    nc.compile()
    return nc


# ---------------- host-side preprocessing ----------------

def _prep_shared(w: dict):
    """Replicated tensors: weights (signed, transposed), thresholds, bn7."""
    out = {}
    w1t = np.sign(w["w1"]).astype(np.float32).transpose(1, 2, 3, 0) \
        .reshape(27, 128).astype(BF16_NP)
    out["w1d"] = np.ascontiguousarray(np.concatenate([w1t, w1t], axis=0))
    out["w1s"] = np.ascontiguousarray(w1t)

    def conv_w(arr, mg, kgr):
        # [O, I, 3, 3] -> [128ki, mg, kg, 9, 128mo] (kg dim dropped if 1)
        O, I = arr.shape[0], arr.shape[1]
        a = np.sign(arr).astype(np.float32).transpose(1, 2, 3, 0)  # I,3,3,O
        a = a.reshape(kgr, 128, 9, mg, 128)        # kg, ki, tap, mg, mo
        a = a.transpose(1, 3, 0, 2, 4)             # ki, mg, kg, tap, mo
        if kgr == 1:
            a = a[:, :, 0]
            if mg == 1:
                a = a[:, 0]
        return np.ascontiguousarray(a.astype(BF16_NP))

    out["w2t"] = conv_w(w["w2"], 1, 1)             # [128, 9, 128]
    out["w3t"] = conv_w(w["w3"], 2, 1)             # [128, 2, 9, 128]
    out["w4t"] = conv_w(w["w4"], 2, 2)
    out["w5t"] = conv_w(w["w5"], 4, 2)
    out["w6t"] = conv_w(w["w6"], 4, 4)

    a7 = np.sign(w["w7"]).astype(np.float32).transpose(1, 2, 3, 0)  # 512,4,4,10
    a7 = a7.reshape(4, 128, 16, 10).transpose(1, 0, 2, 3).reshape(128, 64, 10)
    out["w7t"] = np.ascontiguousarray(a7.astype(BF16_NP))

    thr = np.zeros((128, 14), np.float32)
    f64 = np.float64
    thr[:, 0] = (-(w["bn1_t"].astype(f64) / w["bn1_s"].astype(f64))
                 ).astype(np.float32)
    cols = {2: [1], 3: [2, 3], 4: [4, 5], 5: [6, 7, 8, 9],
            6: [10, 11, 12, 13]}
    for li, cs in cols.items():
        t_ = (-(w[f"bn{li}_t"].astype(f64) /
                (2.0 * w[f"bn{li}_s"].astype(f64)))).astype(np.float32)
        for mi, c in enumerate(cs):
            thr[:, c] = t_[128 * mi:128 * (mi + 1)]
    out["thr"] = thr

    bn7 = np.zeros((10, 2), np.float32)
    bn7[:, 0] = 2.0 * w["bn7_s"]
    bn7[:, 1] = w["bn7_t"]
    out["bn7"] = bn7
    return out


def _prep_x(x_core: np.ndarray):
    """[16,3,32,32] f32 -> 5 bf16 fixed-point planes, shifted per tap,
    stacked pairwise: xa [54,...] (p0,p1), xb [54,...] (p2,p3), xc [27,...]
    (p4).  x == sum(planes) to within 2^-37; each plane is 8-significant-bit
    fixed point, exact in bf16."""
    r = x_core.astype(np.float64)
    planes5 = []
    for i in range(5):
        lsb = 2.0 ** (-4 - 8 * i)
        q = np.round(r / lsb) * lsb
        r = r - q
        planes5.append(q)

    def shifted(arrs):
        out = np.zeros((27 * len(arrs), NIMG, 34 * 34), BF16_NP)
        for pi, a in enumerate(arrs):
            ap = np.pad(a, ((0, 0), (0, 0), (1, 1), (1, 1)))
            base = ap.transpose(1, 0, 2, 3).reshape(3, NIMG, 34 * 34)
            base = base.astype(BF16_NP)
            for c in range(3):
                for dy in range(3):
                    for dx in range(3):
                        k = pi * 27 + c * 9 + dy * 3 + dx
                        s = dy * 34 + dx
                        if s == 0:
                            out[k] = base[c]
                        else:
                            out[k, :, :-s] = base[c, :, s:]
        return out.reshape(27 * len(arrs), NIMG, 34, 34)

    return (shifted(planes5[0:2]), shifted(planes5[2:4]),
            shifted(planes5[4:5]))


def _get_nc():
    global _CACHED_NC
    if _CACHED_NC is None:
        _CACHED_NC = _build_program()
    return _CACHED_NC


def kernel(**inputs):
    inputs = {k: np.asarray(v) for k, v in inputs.items()}
    shared = _prep_shared(inputs)
    x = inputs["x"].astype(np.float32)
    n_total = x.shape[0]
    per = n_total // NCORES

    in_maps = []
    for c in range(NCORES):
        xa, xb, xc = _prep_x(x[c * per:(c + 1) * per])
        m = {"xa": xa, "xb": xb, "xc": xc}
        m.update(shared)
        in_maps.append(m)

    nc = _get_nc()
    last_err = None
    for _ in range(3):  # retry transient NRT device errors
        try:
            res = run_bass_kernel_spmd(nc, in_maps, list(range(NCORES)))
            break
        except Exception as e:  # noqa: BLE001
            last_err = e
    else:
        raise last_err
    outs = [res.results[c]["out"] for c in range(NCORES)]
    return np.concatenate(outs, axis=0).astype(np.float32)



# revision 8
# speedup vs baseline: 2.3142x; 2.3142x over previous
"""Binarized CNN (XNOR-style) inference kernel for Trainium2, 8 NeuronCores.

Strategy
--------
Data parallel: 16 images per core, weights replicated.

All activations/weights after layer 1 are +-1, so convs 2..7 are exact
integer arithmetic.  This version runs them as fp8e4 (e4m3) matmuls in
DoubleRow perf mode: each instruction contracts TWO 128-deep K-planes at
0.5 cycles per output column (4x the bf16 rate).  Key layout choices:

  * DoubleRow moving operands must be [K, 2, N] with N a single strided
    dim, so conv windows are taken full-width over the padded buffer
    ("compute the halo garbage"); threshold/pool ops read only valid
    columns via strided access patterns.
  * Layers with 1 input channel-group (L2, L3) pair TAPS via an
    overlapping stride on the same buffer (odd tap 9 pairs with a
    zero-weight dummy plane at stride 0).  Layers with >=2 channel
    groups (L4..L7) pair channel groups, stored as a free-dim axis.
  * The 8x8-spatial layers (L5, L6) use a 4x4 image mosaic [37,37] with
    shared 1-px halos so windows span 4 images -> 294-wide outputs.
  * Activations alternate encodings: Act-engine Sign produces {-1,+1}
    (layers 1,3,5); GPSIMD is_ge produces {0,1} after the DVE max-pool
    (layers 2,4,6).  Both are exact in fp8.  Per-channel thresholds are
    snapped to the midpoint of the integer conv-sum grid (margin 0.5),
    computed host-side by scanning the reference's fp32 BN transition.
  * Layer 1 needs < 1e-7 accuracy: x decomposes into 4 bf16 fixed-point
    planes (8 significant bits each, lsb 2^-5..2^-32), pre-shifted per
    tap, run as 2 bf16 matmuls whose fp32 PSUM chains are exact.

Engines: PE matmuls ~85us (bottleneck); DVE pools; Act Sign thresholds;
GPSIMD post-pool thresholds + halo memsets; DMA on 4 queues.
"""

import numpy as np
import ml_dtypes

import concourse.bass as bass
import concourse.bacc as bacc_m
import concourse.tile as tile
import concourse.mybir as mybir
from concourse.bass_utils import run_bass_kernel_spmd

F32 = mybir.dt.float32
BF16 = mybir.dt.bfloat16
F8 = mybir.dt.float8e4
BF16_NP = ml_dtypes.bfloat16
F8_NP = ml_dtypes.float8_e4m3fn

NCORES = 8
NIMG = 16
CHUNK = 2
IS_GE = mybir.AluOpType.is_ge
ADD = mybir.AluOpType.add
SUB = mybir.AluOpType.subtract
MULT = mybir.AluOpType.mult
MAX = mybir.AluOpType.max
DR = mybir.MatmulPerfMode.DoubleRow
AXX = mybir.AxisListType.X
AXXY = mybir.AxisListType.XY
SIGN = mybir.ActivationFunctionType.Sign

# weight-table byte offsets (per partition), fp8
W2_OFF = 0
W3_OFF = 1280
W4_OFF = 3840
W5_OFF = 8448
W6_OFF = 17664
W7_OFF = 36096
WTAB_SZ = 36736

# tap pairs for layers with a single input channel-group (9 taps + dummy)
TAP_PAIRS = [(0, 1), (2, 3), (4, 5), (6, 7), (8, None)]

_CACHED_NC = None


def _rhs(t_ap, pstride, off, pair_stride, n):
    return bass.AP(t_ap.tensor, off, [[pstride, 128], [pair_stride, 2], [1, n]])


def _build_program(max_layer=7):
    nc = bacc_m.Bacc(None)

    XA = nc.declare_dram_parameter("xa", [54, NIMG, 34, 34], BF16,
                                   isOutput=False)
    XB = nc.declare_dram_parameter("xb", [54, NIMG, 34, 34], BF16,
                                   isOutput=False)
    W1 = nc.declare_dram_parameter("w1t", [54, 128], BF16, isOutput=False)
    WTAB = nc.declare_dram_parameter("wtab", [128, WTAB_SZ], F8,
                                     isOutput=False)
    THR = nc.declare_dram_parameter("thr", [128, 14], F32, isOutput=False)
    BN7 = nc.declare_dram_parameter("bn7", [10, 2], F32, isOutput=False)
    OUT = nc.declare_dram_parameter("out", [NIMG, 10], F32, isOutput=True)

    with tile.TileContext(nc) as tc:
        with tc.tile_pool(name="w", bufs=1) as wp, \
             tc.tile_pool(name="bp", bufs=1) as bp, \
             tc.tile_pool(name="tmp", bufs=4) as tp, \
             tc.tile_pool(name="psum", bufs=6, space="PSUM") as pp:

            w1t = wp.tile([54, 128], BF16)
            thr = wp.tile([128, 14], F32)
            bn7 = wp.tile([10, 2], F32)
            wtab = wp.tile([128, WTAB_SZ], F8)
            nc.scalar.dma_start(w1t[:], W1[:])
            nc.scalar.dma_start(thr[:], THR[:])
            nc.scalar.dma_start(bn7[:], BN7[:])
            nc.scalar.dma_start(wtab[:], WTAB[:])

            # activation buffers (fp8).  encodings: odd layers' outputs are
            # {-1,+1} w/ halo 0; even layers' outputs {0,1} w/ halo 0.5.
            b1 = bp.tile([128, NIMG, 34, 34], F8)
            b2 = bp.tile([128, NIMG, 18, 18], F8)
            b3 = bp.tile([128, 2, NIMG, 18, 18], F8)
            b4 = bp.tile([128, 2, NIMG, 10, 10], F8)
            b5 = bp.tile([128, 4, NIMG, 10, 10], F8)
            b6 = bp.tile([128, 4, NIMG, 4, 4], F8)
            nc.vector.memset(b1[:, :, 0:34:33, :], 0.0)
            nc.vector.memset(b1[:, :, 1:33, 0:34:33], 0.0)
            nc.vector.memset(b2[:, :, 0:18:17, :], 0.5)
            nc.vector.memset(b2[:, :, 1:17, 0:18:17], 0.5)
            nc.vector.memset(b3[:, :, :, 0:18:17, :], 0.0)
            nc.vector.memset(b3[:, :, :, 1:17, 0:18:17], 0.0)
            nc.vector.memset(b4[:, :, :, 0:10:9, :], 0.5)
            nc.vector.memset(b4[:, :, :, 1:9, 0:10:9], 0.5)
            nc.vector.memset(b5[:, :, :, 0:10:9, :], 0.0)
            nc.vector.memset(b5[:, :, :, 1:9, 0:10:9], 0.0)

            def wslice(off, m):
                return bass.AP(wtab[:].tensor, off,
                               [[WTAB_SZ, 128], [m, 2], [1, m]])

            # ---- L1: exact conv via 4 bf16 fixed-point planes, 2 MMs ----
            with tc.tile_pool(name="xp", bufs=2) as xp:
                for ch in range(NIMG // CHUNK):
                    c0 = ch * CHUNK
                    xa_t = xp.tile([54, CHUNK, 34, 34], BF16, tag="xa")
                    xb_t = xp.tile([54, CHUNK, 34, 34], BF16, tag="xb")
                    nc.sync.dma_start(xa_t[:], XA[:, c0:c0 + CHUNK])
                    nc.gpsimd.dma_start(xb_t[:], XB[:, c0:c0 + CHUNK])
                    for ci in range(CHUNK):
                        n = c0 + ci
                        for h in range(2):
                            ps = pp.tile([128, 512], F32, tag="acc")
                            nc.tensor.matmul(
                                ps[:], w1t[:],
                                xa_t[:, ci, 16 * h:16 * h + 16, 0:32],
                                start=True, stop=False)
                            nc.tensor.matmul(
                                ps[:], w1t[:],
                                xb_t[:, ci, 16 * h:16 * h + 16, 0:32],
                                start=False, stop=True)
                            # Sign(c - thr1) -> +-1
                            nc.scalar.activation(
                                b1[:, n, 1 + 16 * h:17 + 16 * h, 1:33],
                                ps[:].rearrange("p (y x) -> p y x", y=16),
                                SIGN, bias=thr[:, 0:1], scale=1.0)

            b1p = 18496   # partition strides
            b2p = 5184
            b3p = 10368
            b4p = 3200
            b5p = 6400
            b6p = 1024

            # ---- L2: 128->128 (tap pairs), pool, -> b2 {0,1} ----
            for n in range(NIMG if max_layer >= 2 else 0):
                for r0, rb in ((0, 14), (14, 14), (28, 4)):
                    w_ = (rb - 1) * 34 + 32
                    ps = pp.tile([128, 512], F32, tag="acc")
                    for p, (ta, tb) in enumerate(TAP_PAIRS):
                        oa = n * 1156 + (r0 + ta // 3) * 34 + ta % 3
                        d = 0 if tb is None else (
                            (tb // 3 - ta // 3) * 34 + tb % 3 - ta % 3)
                        nc.tensor.matmul(
                            ps[:, 0:w_], wslice(W2_OFF + p * 256, 128),
                            _rhs(b1[:], b1p, oa, d, w_),
                            start=(p == 0), stop=(p == 4), perf_mode=DR)
                    t2 = tp.tile([128, 7, 16], F32, tag="t2")
                    nc.vector.tensor_reduce(
                        t2[:, 0:rb // 2, :],
                        bass.AP(ps[:].tensor, ps[:].offset,
                                [[512, 128], [68, rb // 2], [2, 16],
                                 [34, 2], [1, 2]]),
                        op=MAX, axis=AXXY)
                    nc.gpsimd.tensor_scalar(
                        b2[:, n, 1 + r0 // 2:1 + r0 // 2 + rb // 2, 1:17],
                        t2[:, 0:rb // 2, :], thr[:, 1:2], 0.0, IS_GE, ADD)

            # ---- L3: 128->256 (tap pairs), Sign -> b3 +-1 ----
            for n in range(NIMG if max_layer >= 3 else 0):
                for m in range(2):
                    ps = pp.tile([128, 512], F32, tag="acc")
                    for p, (ta, tb) in enumerate(TAP_PAIRS):
                        oa = n * 324 + (ta // 3) * 18 + ta % 3
                        d = 0 if tb is None else (
                            (tb // 3 - ta // 3) * 18 + tb % 3 - ta % 3)
                        nc.tensor.matmul(
                            ps[:, 0:286],
                            wslice(W3_OFF + (m * 5 + p) * 256, 128),
                            _rhs(b2[:], b2p, oa, d, 286),
                            start=(p == 0), stop=(p == 4), perf_mode=DR)
                    nc.scalar.activation(
                        b3[:, m, n, 1:17, 1:17],
                        bass.AP(ps[:].tensor, ps[:].offset,
                                [[512, 128], [18, 16], [1, 16]]),
                        SIGN, bias=thr[:, 2 + m:3 + m], scale=1.0)

            # ---- L4: 256->256 (kg pairs), pool, -> b4 mosaic {0,1} ----
            for n in range(NIMG if max_layer >= 4 else 0):
                for m in range(2):
                    ps = pp.tile([128, 512], F32, tag="acc")
                    for t in range(9):
                        o = n * 324 + (t // 3) * 18 + t % 3
                        nc.tensor.matmul(
                            ps[:, 0:286],
                            wslice(W4_OFF + (m * 9 + t) * 256, 128),
                            _rhs(b3[:], b3p, o, 5184, 286),
                            start=(t == 0), stop=(t == 8), perf_mode=DR)
                    t4 = tp.tile([128, 8, 8], F32, tag="t4")
                    nc.vector.tensor_reduce(
                        t4[:],
                        bass.AP(ps[:].tensor, ps[:].offset,
                                [[512, 128], [36, 8], [2, 8],
                                 [18, 2], [1, 2]]),
                        op=MAX, axis=AXXY)
                    nc.gpsimd.tensor_scalar(
                        b4[:, m, n, 1:9, 1:9],
                        t4[:], thr[:, 4 + m:5 + m], 0.0, IS_GE, ADD)

            # ---- L5: 256->512 (kg pairs), Sign -> b5 +-1 ----
            for n in range(NIMG if max_layer >= 5 else 0):
                for m in range(4):
                    ps = pp.tile([128, 512], F32, tag="acc")
                    for t in range(9):
                        o = n * 100 + (t // 3) * 10 + t % 3
                        nc.tensor.matmul(
                            ps[:, 0:78],
                            wslice(W5_OFF + (m * 9 + t) * 256, 128),
                            _rhs(b4[:], b4p, o, 1600, 78),
                            start=(t == 0), stop=(t == 8), perf_mode=DR)
                    nc.scalar.activation(
                        b5[:, m, n, 1:9, 1:9],
                        bass.AP(ps[:].tensor, ps[:].offset,
                                [[512, 128], [10, 8], [1, 8]]),
                        SIGN, bias=thr[:, 6 + m:7 + m], scale=1.0)

            # ---- L6: 512->512 (kg pairs), pool, -> b6 {0,1} ----
            for n in range(NIMG if max_layer >= 6 else 0):
                for m in range(4):
                    ps = pp.tile([128, 512], F32, tag="acc")
                    idx = 0
                    for kp in range(2):
                        for t in range(9):
                            o = kp * 3200 + n * 100 + (t // 3) * 10 + t % 3
                            nc.tensor.matmul(
                                ps[:, 0:78],
                                wslice(W6_OFF + (m * 18 + kp * 9 + t) * 256,
                                       128),
                                _rhs(b5[:], b5p, o, 1600, 78),
                                start=(idx == 0), stop=(idx == 17),
                                perf_mode=DR)
                            idx += 1
                    t6 = tp.tile([128, 4, 4], F32, tag="t6")
                    nc.vector.tensor_reduce(
                        t6[:],
                        bass.AP(ps[:].tensor, ps[:].offset,
                                [[512, 128], [20, 4], [2, 4],
                                 [10, 2], [1, 2]]),
                        op=MAX, axis=AXXY)
                    nc.gpsimd.tensor_scalar(
                        b6[:, m, n, :, :],
                        t6[:], thr[:, 10 + m:11 + m], 0.0, IS_GE, ADD)

            # ---- L7: 512x4x4 -> 10 logits (plain fp8, 64 matmuls) ----
            ps7 = pp.tile([10, 16], F32, tag="z7", bufs=1)
            if max_layer < 7:
                nc.vector.memset(ps7[:], 0.0)
            for j in range(64 if max_layer >= 7 else 0):
                kg, t16 = divmod(j, 16)
                nc.tensor.matmul(
                    ps7[:],
                    bass.AP(wtab[:].tensor, W7_OFF + j * 10,
                            [[WTAB_SZ, 128], [1, 10]]),
                    bass.AP(b6[:].tensor, kg * 256 + (t16 // 4) * 4 + t16 % 4,
                            [[b6p, 128], [16, 16]]),
                    start=(j == 0), stop=(j == 63))

            # ---- BN7 + log_softmax ----
            sq = tp.tile([32, 32], F32, tag="sq")
            nc.vector.memset(sq[:], 0.0)
            nc.vector.tensor_scalar(sq[0:10, 0:16], ps7[:], bn7[:, 0:1],
                                    bn7[:, 1:2], MULT, ADD)
            tq = tp.tile([32, 32], F32, tag="tq")
            nc.vector.transpose(tq[:], sq[:])
            yt = tq[0:16, 0:10]
            nm = tp.tile([16, 1], F32, tag="nm")
            nc.vector.tensor_reduce(nm[:], yt, op=MAX, axis=AXX, negate=True)
            e = tp.tile([16, 10], F32, tag="e")
            nc.scalar.activation(e[:], yt, mybir.ActivationFunctionType.Exp,
                                 bias=nm[:], scale=1.0)
            s_ = tp.tile([16, 1], F32, tag="S")
            nc.vector.tensor_reduce(s_[:], e[:], op=ADD, axis=AXX)
            lns = tp.tile([16, 1], F32, tag="lnS")
            nc.scalar.activation(lns[:], s_[:],
                                 mybir.ActivationFunctionType.Ln)
            o = tp.tile([16, 10], F32, tag="o")
            nc.vector.tensor_scalar(o[:], yt, nm[:], lns[:], ADD, SUB)
            nc.sync.dma_start(OUT[:], o[:])

    nc.compile()
    return nc


# ---------------- host-side preprocessing ----------------

def _thr_mid(s, t, kmax):
    """Midpoint threshold on the integer conv-sum grid: c* - 0.5 where c* is
    the first integer c with fl32(fl32(c*s)+t) > 0 (reference decision)."""
    s32 = np.asarray(s, np.float32)
    t32 = np.asarray(t, np.float32)
    c = np.arange(-kmax, kmax + 2, dtype=np.float32)
    v = s32[:, None] * c[None, :] + t32[:, None]
    pos = v > 0
    first = pos.argmax(axis=1)
    cstar = c[first].astype(np.float64)
    cstar[~pos.any(axis=1)] = kmax + 1
    return cstar - 0.5


def _prep_shared(w):
    out = {}
    w1t = np.sign(w["w1"]).astype(np.float32).transpose(1, 2, 3, 0) \
        .reshape(27, 128)
    out["w1t"] = np.ascontiguousarray(
        np.concatenate([w1t, w1t], axis=0).astype(BF16_NP))

    tab = np.zeros((128, WTAB_SZ), F8_NP)

    def sgn(a):
        return np.sign(a).astype(np.float32)

    # L2/L3: [ki, pair, i, mo] tap pairs, dummy plane zero
    s2 = sgn(w["w2"]).transpose(1, 2, 3, 0).reshape(128, 9, 128)
    for p, (ta, tb) in enumerate(TAP_PAIRS):
        o = W2_OFF + p * 256
        tab[:, o:o + 128] = s2[:, ta, :]
        if tb is not None:
            tab[:, o + 128:o + 256] = s2[:, tb, :]
    s3 = sgn(w["w3"]).transpose(1, 2, 3, 0).reshape(128, 9, 256)
    for m in range(2):
        for p, (ta, tb) in enumerate(TAP_PAIRS):
            o = W3_OFF + (m * 5 + p) * 256
            tab[:, o:o + 128] = s3[:, ta, 128 * m:128 * m + 128]
            if tb is not None:
                tab[:, o + 128:o + 256] = s3[:, tb, 128 * m:128 * m + 128]

    # L4/L5: [ki, m, tap, kg, mo] channel-group pairs
    s4 = sgn(w["w4"]).transpose(1, 2, 3, 0).reshape(2, 128, 9, 256)
    for m in range(2):
        for t in range(9):
            o = W4_OFF + (m * 9 + t) * 256
            for i in range(2):
                tab[:, o + 128 * i:o + 128 * (i + 1)] = \
                    s4[i, :, t, 128 * m:128 * m + 128]
    s5 = sgn(w["w5"]).transpose(1, 2, 3, 0).reshape(2, 128, 9, 512)
    for m in range(4):
        for t in range(9):
            o = W5_OFF + (m * 9 + t) * 256
            for i in range(2):
                tab[:, o + 128 * i:o + 128 * (i + 1)] = \
                    s5[i, :, t, 128 * m:128 * m + 128]
    s6 = sgn(w["w6"]).transpose(1, 2, 3, 0).reshape(4, 128, 9, 512)
    for m in range(4):
        for kp in range(2):
            for t in range(9):
                o = W6_OFF + (m * 18 + kp * 9 + t) * 256
                for i in range(2):
                    tab[:, o + 128 * i:o + 128 * (i + 1)] = \
                        s6[2 * kp + i, :, t, 128 * m:128 * m + 128]
    # L7: [ki, j=kp*16+t16, i, 10]
    s7 = sgn(w["w7"]).transpose(1, 2, 3, 0).reshape(4, 128, 16, 10)
    for j in range(64):
        kg, t16 = divmod(j, 16)
        o = W7_OFF + j * 10
        tab[:, o:o + 10] = s7[kg, :, t16, :]
    out["wtab"] = tab

    # thresholds.  input encodings: b1,b3,b5 are +-1 (alpha=1, beta=0);
    # b2,b4 are {0,1} (alpha=.5, beta=.5).  thr_enc = a*thr_mid + b*Kw.
    f64 = np.float64
    thr = np.zeros((128, 14), np.float32)
    # L1: Act Sign bias = -thr1 = t/s (f64->f32)
    thr[:, 0] = (w["bn1_t"].astype(f64) / w["bn1_s"].astype(f64)
                 ).astype(np.float32)

    def kw(key):
        a = np.sign(w[key]).astype(f64)
        return a.reshape(a.shape[0], -1).sum(axis=1)

    tm2 = _thr_mid(w["bn2_s"], w["bn2_t"], 1152)
    thr[:, 1] = tm2.astype(np.float32)                      # is_ge, +-1 in
    tm3 = _thr_mid(w["bn3_s"], w["bn3_t"], 1152)
    t3 = -(0.5 * tm3 + 0.5 * kw("w3"))                      # Sign bias
    thr[:, 2] = t3[0:128].astype(np.float32)
    thr[:, 3] = t3[128:256].astype(np.float32)
    tm4 = _thr_mid(w["bn4_s"], w["bn4_t"], 2304)
    thr[:, 4] = tm4[0:128].astype(np.float32)
    thr[:, 5] = tm4[128:256].astype(np.float32)
    tm5 = _thr_mid(w["bn5_s"], w["bn5_t"], 2304)
    t5 = -(0.5 * tm5 + 0.5 * kw("w5"))
    for m in range(4):
        thr[:, 6 + m] = t5[128 * m:128 * m + 128].astype(np.float32)
    tm6 = _thr_mid(w["bn6_s"], w["bn6_t"], 4608)
    for m in range(4):
        thr[:, 10 + m] = tm6[128 * m:128 * m + 128].astype(np.float32)
    out["thr"] = thr

    # L7 input b6 is {0,1}: y = c01*(2*s7) + (t7 - Kw7*s7)
    bn7 = np.zeros((10, 2), np.float32)
    bn7[:, 0] = 2.0 * w["bn7_s"]
    bn7[:, 1] = (w["bn7_t"].astype(f64)
                 - kw("w7") * w["bn7_s"].astype(f64)).astype(np.float32)
    out["bn7"] = bn7
    return out


def _prep_x(x_core):
    """[16,3,32,32] f32 -> 4 bf16 fixed-point planes (lsb 2^-5..2^-32),
    pre-shifted per tap, pair-stacked: xa rows = planes 0,1; xb = 2,3."""
    r = x_core.astype(np.float64)
    planes = []
    for i in range(4):
        lsb = 2.0 ** (-5 - 9 * i)
        q = np.round(r / lsb) * lsb
        r = r - q
        planes.append(q)

    def shifted(arrs):
        o = np.zeros((27 * len(arrs), NIMG, 34 * 34), BF16_NP)
        for pi, a in enumerate(arrs):
            ap = np.pad(a, ((0, 0), (0, 0), (1, 1), (1, 1)))
            base = ap.transpose(1, 0, 2, 3).reshape(3, NIMG, 34 * 34)
            base = base.astype(BF16_NP)
            for c in range(3):
                for dy in range(3):
                    for dx in range(3):
                        k = pi * 27 + c * 9 + dy * 3 + dx
                        s = dy * 34 + dx
                        if s == 0:
                            o[k] = base[c]
                        else:
                            o[k, :, :-s] = base[c, :, s:]
        return o.reshape(27 * len(arrs), NIMG, 34, 34)

    return shifted(planes[0:2]), shifted(planes[2:4])


def _get_nc():
    global _CACHED_NC
    if _CACHED_NC is None:
        _CACHED_NC = _build_program()
    return _CACHED_NC


def kernel(**inputs):
    inputs = {k: np.asarray(v) for k, v in inputs.items()}
    shared = _prep_shared(inputs)
    x = inputs["x"].astype(np.float32)
    per = x.shape[0] // NCORES

    in_maps = []
    for c in range(NCORES):
        xa, xb = _prep_x(x[c * per:(c + 1) * per])
        m = {"xa": xa, "xb": xb}
        m.update(shared)
        in_maps.append(m)

    nc = _get_nc()
    last_err = None
    for _ in range(3):  # retry transient NRT device errors
        try:
            res = run_bass_kernel_spmd(nc, in_maps, list(range(NCORES)))
            break
        except Exception as e:  # noqa: BLE001
            last_err = e
    else:
        raise last_err
    outs = [res.results[c]["out"] for c in range(NCORES)]
    return np.concatenate(outs, axis=0).astype(np.float32)
